# revision 13
# baseline (speedup 1.0000x reference)
import os
import sys

sys.path.insert(0, "/opt/trn_rl_repo")
os.environ.setdefault("MYCRO_LOCAL_CACHE", "1")

import numpy as np

B, N, CIN = 8, 4096, 6
S, K = 1024, 32
STRIDE = N // S            # 4
P = 128
NCHUNK = S // P            # 8 chunks of 128 centers
NBLK = N // P              # 32 column blocks of 128 points
NT = S * K // 512          # 64 tiles of 512 cols for the MLP phases (32768 cols)
BN_EPS = 1e-5
NEG_BIG = -1.0e30
NCORES = 8

_CACHE = {}


def _build_program(no_cc=False):
    from concourse import bass, tile, mybir

    f32 = mybir.dt.float32
    bf16 = mybir.dt.bfloat16
    i16 = mybir.dt.int16
    u16 = mybir.dt.uint16
    Alu = mybir.AluOpType
    Act = mybir.ActivationFunctionType
    Ax = mybir.AxisListType

    nc = bass.Bass("TRN2", target_bir_lowering=False, debug=False,
                   num_devices=NCORES)

    feats = nc.dram_tensor("feats9T", [9, N], f32, kind="ExternalInput").ap()
    twocT = nc.dram_tensor("twocT", [3, S], f32, kind="ExternalInput").ap()
    cT = nc.dram_tensor("cT", [3, S], f32, kind="ExternalInput").ap()
    A8d = nc.dram_tensor("A8", [P, NCHUNK], f32, kind="ExternalInput").ap()
    negBd = nc.dram_tensor("negB", [P, N], f32, kind="ExternalInput").ap()
    offsd = nc.dram_tensor("offs", [P, NBLK * 8], f32, kind="ExternalInput").ap()
    W0Td = nc.dram_tensor("W0T", [9, 64], f32, kind="ExternalInput").ap()
    W0aTd = nc.dram_tensor("W0aT", [3, 64], f32, kind="ExternalInput").ap()
    W1Td = nc.dram_tensor("W1T", [64, 64], f32, kind="ExternalInput").ap()
    W2Td = nc.dram_tensor("W2T", [64, 128], f32, kind="ExternalInput").ap()
    gb0d = nc.dram_tensor("gb0", [64, 2], f32, kind="ExternalInput").ap()
    gb1d = nc.dram_tensor("gb1", [64, 2], f32, kind="ExternalInput").ap()
    gb2d = nc.dram_tensor("gb2", [128, 2], f32, kind="ExternalInput").ap()
    identd = nc.dram_tensor("ident", [P, P], f32, kind="ExternalInput").ap()
    out_pts = nc.dram_tensor("out_pts", [S, 128], f32, kind="ExternalOutput").ap()
    out_idx = nc.dram_tensor("out_idx", [S, K], i16, kind="ExternalOutput").ap()

    with tile.TileContext(nc) as tc:
        from concourse import library_config
        nc.gpsimd.load_library(library_config.ap_gather)
        with tc.tile_pool(name="glob", bufs=1) as gp, \
             tc.tile_pool(name="gdram", bufs=1, space="DRAM") as gdp:

            y0_sb = gp.tile([64, S * K], bf16, name="y0_sb", tag="y0_sb")
            stats0 = gp.tile([64, NT * 6], f32, name="stats0", tag="stats0")
            stats1 = gp.tile([64, NT * 6], f32, name="stats1", tag="stats1")
            stats2 = gp.tile([128, NT * 6], f32, name="stats2", tag="stats2")
            W1T = gp.tile([64, 64], f32, name="W1T", tag="W1T")
            W2T = gp.tile([64, 128], f32, name="W2T", tag="W2T")
            gb0 = gp.tile([64, 2], f32, name="gb0", tag="gb0")
            gb1 = gp.tile([64, 2], f32, name="gb1", tag="gb1")
            gb2 = gp.tile([128, 2], f32, name="gb2", tag="gb2")
            ident = gp.tile([P, P], f32, name="ident", tag="ident")
            pooled = gp.tile([128, S], f32, name="pooled", tag="pooled")

            nc.sync.dma_start(W1T, W1Td)
            nc.sync.dma_start(W2T, W2Td)
            nc.sync.dma_start(gb0, gb0d)
            nc.sync.dma_start(gb1, gb1d)
            nc.sync.dma_start(gb2, gb2d)
            nc.sync.dma_start(ident, identd)

            def bn_ar(stats_ap, gb_ap, pdim, tag):
                """Aggregate local bn stats, AllReduce (mean, E[x^2]),
                produce ab tile [pdim,2] = (alpha, beta)."""
                mv = gp.tile([pdim, 2], f32, name=f"mv_{tag}", tag=f"mv_{tag}")
                nc.vector.bn_aggr(mv, stats_ap)
                pl = gp.tile([pdim, 2], f32, name=f"pl_{tag}", tag=f"pl_{tag}")
                nc.vector.tensor_copy(pl[:, 0:1], mv[:, 0:1])
                nc.vector.tensor_tensor(pl[:, 1:2], mv[:, 0:1], mv[:, 0:1],
                                        Alu.mult)
                nc.vector.tensor_tensor(pl[:, 1:2], pl[:, 1:2], mv[:, 1:2],
                                        Alu.add)
                bin_ = gdp.tile([pdim, 2], f32, name=f"bin_{tag}",
                                tag=f"bin_{tag}")
                bout = gdp.tile([pdim, 2], f32, name=f"bout_{tag}",
                                tag=f"bout_{tag}")
                nc.sync.dma_start(bin_, pl)
                if no_cc:
                    nc.sync.dma_start(bout, bin_)
                else:
                    nc.gpsimd.collective_compute(
                        "AllReduce", Alu.add,
                        replica_groups=[list(range(NCORES))],
                        ins=[bin_.opt()], outs=[bout.opt()])
                ar = gp.tile([pdim, 2], f32, name=f"ar_{tag}", tag=f"ar_{tag}")
                nc.sync.dma_start(ar, bout)
                w = gp.tile([pdim, 4], f32, name=f"w_{tag}", tag=f"w_{tag}")
                # w cols: 0=m, 1=ex2, 2=v then 1/sqrt(v+eps), 3=m*alpha
                nc.vector.tensor_scalar(w[:, 0:1], ar[:, 0:1], 1.0 / NCORES,
                                        None, Alu.mult)
                nc.vector.tensor_scalar(w[:, 1:2], ar[:, 1:2], 1.0 / NCORES,
                                        None, Alu.mult)
                nc.vector.tensor_tensor(w[:, 2:3], w[:, 0:1], w[:, 0:1],
                                        Alu.mult)
                nc.vector.tensor_tensor(w[:, 2:3], w[:, 1:2], w[:, 2:3],
                                        Alu.subtract)
                sq = gp.tile([pdim, 2], f32, name=f"sq_{tag}", tag=f"sq_{tag}")
                nc.vector.tensor_scalar(sq[:, 0:1], w[:, 2:3], float(BN_EPS),
                                        None, Alu.add)
                nc.scalar.activation(sq[:, 1:2], sq[:, 0:1], Act.Sqrt)
                nc.vector.reciprocal(w[:, 2:3], sq[:, 1:2])
                ab = gp.tile([pdim, 2], f32, name=f"ab_{tag}", tag=f"ab_{tag}")
                nc.vector.tensor_tensor(ab[:, 0:1], gb_ap[:, 0:1], w[:, 2:3],
                                        Alu.mult)
                nc.vector.tensor_tensor(w[:, 3:4], w[:, 0:1], ab[:, 0:1],
                                        Alu.mult)
                nc.vector.tensor_tensor(ab[:, 1:2], gb_ap[:, 1:2], w[:, 3:4],
                                        Alu.subtract)
                return ab

            # ---------------- Phase 0 + 1: scores, top-32, gather, conv0 ----
            with tc.tile_pool(name="p1", bufs=1) as p1, \
                 tc.tile_pool(name="ps1", bufs=2, space="PSUM") as ps1:

                featsb = p1.tile([9, N], f32, name="featsb", tag="featsb")
                W0T = p1.tile([9, 64], f32, name="W0T", tag="W0T")
                W0aT = p1.tile([3, 64], f32, name="W0aT", tag="W0aT")
                twocb = p1.tile([3, S], f32, name="twocb", tag="twocb")
                cb = p1.tile([3, S], f32, name="cb", tag="cb")
                A8 = p1.tile([P, NCHUNK], f32, name="A8", tag="A8")
                negB = p1.tile([P, N], f32, name="negB", tag="negB")
                offs = p1.tile([P, NBLK * 8], f32, name="offs", tag="offs")
                u = p1.tile([64, N], f32, name="u", tag="u")
                c0 = p1.tile([64, S], f32, name="c0", tag="c0")

                nc.sync.dma_start(featsb, feats)
                nc.sync.dma_start(W0T, W0Td)
                nc.sync.dma_start(W0aT, W0aTd)
                nc.sync.dma_start(twocb, twocT)
                nc.sync.dma_start(cb, cT)
                nc.sync.dma_start(A8, A8d)
                nc.sync.dma_start(negB, negBd)
                nc.sync.dma_start(offs, offsd)

                # u = W0 @ feats  [64, N]
                for j in range(N // 512):
                    mp = ps1.tile([64, 512], f32, name="mp0", tag="mm0")
                    nc.tensor.matmul(mp, W0T, featsb[:, j * 512:(j + 1) * 512],
                                     start=True, stop=True)
                    nc.scalar.activation(u[:, j * 512:(j + 1) * 512], mp,
                                         Act.Copy)
                # c0 = W0[:, :3] @ c^T  [64, S]
                for j in range(S // 512):
                    mp = ps1.tile([64, 512], f32, name="mpc", tag="mm0")
                    nc.tensor.matmul(mp, W0aT, cb[:, j * 512:(j + 1) * 512],
                                     start=True, stop=True)
                    nc.scalar.activation(c0[:, j * 512:(j + 1) * 512], mp,
                                         Act.Copy)

                for ch in range(NCHUNK):
                    s_sb = p1.tile([P, N], f32, name="s_sb", tag="s_sb")
                    # scores s = (negB - A) + 2c.p   (= -d bitwise)
                    for j in range(N // 512):
                        e2p = ps1.tile([P, 512], f32, name="e2p", tag="e2")
                        nc.tensor.matmul(
                            e2p, twocb[:, ch * P:(ch + 1) * P],
                            featsb[0:3, j * 512:(j + 1) * 512],
                            start=True, stop=True)
                        nc.vector.scalar_tensor_tensor(
                            s_sb[:, j * 512:(j + 1) * 512],
                            negB[:, j * 512:(j + 1) * 512],
                            A8[:, ch:ch + 1], e2p,
                            Alu.subtract, Alu.add)

                    cand_val = p1.tile([P, NBLK * 8], f32, name="cand_val",
                                       tag="cand_val")
                    cand_idx = p1.tile([P, NBLK * 8], u16, name="cand_idx",
                                       tag="cand_idx")
                    for blk in range(NBLK):
                        cv = cand_val[:, blk * 8:(blk + 1) * 8]
                        sb = s_sb[:, blk * P:(blk + 1) * P]
                        nc.vector.max(cv, sb)
                        nc.vector.max_index(
                            cand_idx[:, blk * 8:(blk + 1) * 8], cv, sb)

                    gidx = p1.tile([P, NBLK * 8], f32, name="gidx", tag="gidx")
                    graw = p1.tile([P, NBLK * 8], f32, name="graw", tag="graw")
                    nc.vector.tensor_copy(graw, cand_idx)
                    nc.vector.tensor_tensor(gidx, graw, offs, Alu.add)

                    # knock out the global top-32 values among candidates
                    va = p1.tile([P, NBLK * 8], f32, name="va", tag="va")
                    vb = p1.tile([P, NBLK * 8], f32, name="vb", tag="vb")
                    v32 = p1.tile([P, K], f32, name="v32", tag="v32")
                    nc.vector.tensor_copy(va, cand_val)
                    cur, nxt = va, vb
                    for r in range(4):
                        e8 = v32[:, r * 8:(r + 1) * 8]
                        nc.vector.max(e8, cur)
                        nc.vector.match_replace(nxt, e8, cur, NEG_BIG)
                        cur, nxt = nxt, cur
                    # cur now has NEG_BIG exactly at the selected 32 positions
                    pred = p1.tile([P, NBLK * 8], f32, name="pred", tag="pred")
                    nc.vector.tensor_scalar(pred, cur, NEG_BIG, None,
                                            Alu.is_equal)
                    tmpa = p1.tile([P, NBLK * 8], f32, name="tmpa", tag="tmpa")
                    tmpb = p1.tile([P, NBLK * 8], f32, name="tmpb", tag="tmpb")
                    nc.vector.tensor_tensor(tmpa, pred, gidx, Alu.mult)
                    idxp1 = p1.tile([P, K], f32, name="idxp1", tag="idxp1")
                    cur, nxt = tmpa, tmpb
                    for r in range(4):
                        e8 = idxp1[:, r * 8:(r + 1) * 8]
                        nc.vector.max(e8, cur)
                        if r < 3:
                            nc.vector.match_replace(nxt, e8, cur, 0.0)
                            cur, nxt = nxt, cur
                    idx_i16 = p1.tile([P, K], i16, name="idx_i16",
                                      tag="idx_i16")
                    nc.vector.tensor_scalar(idx_i16, idxp1, -1.0, None,
                                            Alu.add)

                    # DRAM round-trip to build the 16-partition-wrapped index
                    # layout ap_gather wants, replicated for 4 gpsimd cores.
                    oslice = out_idx[ch * P:(ch + 1) * P, :]
                    nc.sync.dma_start(oslice, idx_i16)
                    wr_idx = p1.tile([64, P * K // 16], i16, name="wr_idx",
                                     tag="wr_idx")
                    src = oslice.rearrange("m (r p) -> p m r", p=16)
                    for g in range(4):
                        dst = wr_idx[g * 16:(g + 1) * 16, :].rearrange(
                            "p (m r) -> p m r", r=2)
                        nc.sync.dma_start(dst, src)

                    u_g = p1.tile([64, P * K], f32, name="u_g", tag="u_g")
                    nc.gpsimd.ap_gather(u_g, u, wr_idx, channels=64,
                                        num_elems=N, d=1, num_idxs=P * K)

                    # y0 = u[idx] - c0[center]
                    y0t = p1.tile([64, P * K], f32, name="y0t", tag="y0t")
                    c0b = c0[:, ch * P:(ch + 1) * P].rearrange(
                        "c (m o) -> c m o", o=1).broadcast_to((64, P, K))
                    nc.vector.tensor_tensor(
                        y0t.rearrange("c (m k) -> c m k", k=K),
                        u_g.rearrange("c (m k) -> c m k", k=K),
                        c0b, Alu.subtract)
                    for j in range(P * K // 512):
                        t = ch * (P * K // 512) + j
                        nc.vector.bn_stats(stats0[:, t * 6:(t + 1) * 6],
                                           y0t[:, j * 512:(j + 1) * 512])
                    nc.scalar.activation(
                        y0_sb[:, ch * P * K:(ch + 1) * P * K],
                        y0t, Act.Copy)

            # ---------------- Phase 2 + 3: conv1, conv2, maxpool ------------
            ab0 = bn_ar(stats0, gb0, 64, "bn0")
            with tc.tile_pool(name="p23", bufs=1) as p23, \
                 tc.tile_pool(name="ps23", bufs=2, space="PSUM") as ps23:
                y1_sb = p23.tile([64, S * K], bf16, name="y1_sb",
                                 tag="y1_sb")
                for t in range(NT):
                    x0 = p23.tile([64, 512], f32, name="x0", tag="x0", bufs=2)
                    nc.scalar.activation(x0,
                                         y0_sb[:, t * 512:(t + 1) * 512],
                                         Act.Relu, bias=ab0[:, 1:2],
                                         scale=ab0[:, 0:1])
                    h1p = ps23.tile([64, 512], f32, name="h1p", tag="h1")
                    nc.tensor.matmul(h1p, W1T, x0, start=True, stop=True)
                    nc.vector.bn_stats(stats1[:, t * 6:(t + 1) * 6], h1p)
                    nc.scalar.activation(y1_sb[:, t * 512:(t + 1) * 512],
                                         h1p, Act.Copy)

                ab1 = bn_ar(stats1, gb1, 64, "bn1")
                for t in range(NT):
                    x1 = p23.tile([64, 512], f32, name="x1", tag="x1", bufs=2)
                    nc.scalar.activation(x1,
                                         y1_sb[:, t * 512:(t + 1) * 512],
                                         Act.Relu, bias=ab1[:, 1:2],
                                         scale=ab1[:, 0:1])
                    h2p = ps23.tile([128, 512], f32, name="h2p", tag="h2")
                    nc.tensor.matmul(h2p, W2T, x1, start=True, stop=True)
                    nc.vector.bn_stats(stats2[:, t * 6:(t + 1) * 6], h2p)
                    nc.vector.tensor_reduce(
                        pooled[:, t * 16:(t + 1) * 16].rearrange(
                            "p (m o) -> p m o", o=1),
                        h2p.rearrange("p (m k) -> p m k", k=K),
                        Ax.X, Alu.max)

            # ---------------- Phase 4: BN2 affine + relu, transpose out -----
            ab2 = bn_ar(stats2, gb2, 128, "bn2")
            with tc.tile_pool(name="p4", bufs=2) as p4, \
                 tc.tile_pool(name="ps4", bufs=2, space="PSUM") as ps4:
                out_sb = p4.tile([128, S], f32, name="out_sb", tag="out_sb",
                                 bufs=1)
                nc.scalar.activation(out_sb, pooled, Act.Relu,
                                     bias=ab2[:, 1:2], scale=ab2[:, 0:1])
                for j in range(S // P):
                    tp = ps4.tile([P, P], f32, name="tp", tag="tp")
                    nc.tensor.transpose(tp, out_sb[:, j * P:(j + 1) * P],
                                        ident)
                    oT = p4.tile([P, P], f32, name="oT", tag="oT")
                    nc.scalar.activation(oT, tp, Act.Copy)
                    nc.sync.dma_start(out_pts[j * P:(j + 1) * P, :], oT)

    import bass_rust as _bass_rust
    from concourse import library_overlay
    _bass_rust.move_matmul_waits_to_ldweights(nc.m)
    _bass_rust.generate_event_semaphores(nc)
    library_overlay.lower_extended_insts(nc)
    return nc


def _get_program():
    if "nc" not in _CACHE:
        _CACHE["nc"] = _build_program()
    return _CACHE["nc"]


def _host_inputs(xyz, points, W0, g0, b0, W1, g1, b1, W2, g2, b2):
    f32 = np.float32
    shared = {}
    W0 = W0.astype(f32)
    shared["W0T"] = np.ascontiguousarray(W0.T)
    shared["W0aT"] = np.ascontiguousarray(W0[:, :3].T)
    shared["W1T"] = np.ascontiguousarray(W1.astype(f32).T)
    shared["W2T"] = np.ascontiguousarray(W2.astype(f32).T)
    shared["gb0"] = np.ascontiguousarray(np.stack([g0, b0], 1).astype(f32))
    shared["gb1"] = np.ascontiguousarray(np.stack([g1, b1], 1).astype(f32))
    shared["gb2"] = np.ascontiguousarray(np.stack([g2, b2], 1).astype(f32))
    shared["ident"] = np.eye(P, dtype=f32)
    offs = (np.repeat(np.arange(NBLK, dtype=f32) * P, 8) + f32(1.0))
    shared["offs"] = np.ascontiguousarray(
        np.broadcast_to(offs.astype(f32), (P, NBLK * 8)))

    in_maps = []
    for b in range(NCORES):
        p = xyz[b].astype(f32)
        c = p[::STRIDE]
        # host A/B in the exact fp32 op order that bitwise-matches XLA
        A = ((c[:, 0] * c[:, 0] + c[:, 1] * c[:, 1]) + c[:, 2] * c[:, 2])
        A = A.astype(f32)
        negB = -((p[:, 0] * p[:, 0] + p[:, 1] * p[:, 1])
                 + p[:, 2] * p[:, 2]).astype(f32)
        m = dict(shared)
        m["feats9T"] = np.ascontiguousarray(
            np.concatenate([p, points[b].astype(f32)], 1).T)
        m["twocT"] = np.ascontiguousarray((f32(2.0) * c).T)
        m["cT"] = np.ascontiguousarray(c.T)
        m["A8"] = np.ascontiguousarray(A.reshape(NCHUNK, P).T)
        m["negB"] = np.ascontiguousarray(np.broadcast_to(negB, (P, N)))
        in_maps.append(m)
    return in_maps


def kernel(**inputs):
    xyz = np.asarray(inputs["xyz"], dtype=np.float32)
    points = np.asarray(inputs["points"], dtype=np.float32)
    args = [np.asarray(inputs[k], dtype=np.float32)
            for k in ("W0", "g0", "b0", "W1", "g1", "b1", "W2", "g2", "b2")]
    in_maps = _host_inputs(xyz, points, *args)
    nc = _get_program()
    from concourse import bass_utils
    res = bass_utils.run_bass_kernel_spmd(nc, in_maps,
                                          core_ids=list(range(NCORES)))
    _CACHE["last_results"] = res.results
    new_points = np.stack(
        [np.asarray(res.results[b]["out_pts"]) for b in range(NCORES)])
    new_xyz = np.ascontiguousarray(xyz[:, ::STRIDE])
    return new_xyz, new_points
